# revision 24
# baseline (speedup 1.0000x reference)
"""Bahdanau additive attention on 8 TRN2 NeuronCores.

Reference computation (B=1, S=65536, H=512, A=256):
    si_proj = si @ Wa.T                       [S, A]
    enc_proj = enc_out @ Ua.T                 [A]
    e = tanh(si_proj + enc_proj) @ va         [S]
    alpha = softmax(e)                        [S]
    context = alpha @ si                      [H]
    returns (context [1,H], alpha [1,S,1])

Distribution: sequence-parallel over S. Each core owns 8192 rows of si,
computes its local unnormalized exp(e) and partial weighted sums, then one
513-float AllReduce combines the softmax normalizer Z and partial context P.
alpha = exp(e - ln Z) and context = P/Z are finalized on-device.

softmax max-subtraction is skipped: |e| <= sum|va| ~ 13, exp(13)*65536 fits
comfortably in f32.
"""

import sys
import numpy as np

for _p in ("/opt/trn_rl_repo",):
    if _p not in sys.path:
        sys.path.insert(0, _p)

import concourse.bass as bass
import concourse.bacc as bacc
import concourse.mybir as mybir
from concourse.tile import TileContext
from concourse.tile_rust import add_dep_helper
from concourse.bass_utils import run_bass_kernel_spmd
from concourse.masks import make_identity

F32 = mybir.dt.float32
BF16 = mybir.dt.bfloat16

B, S, H, A = 1, 65536, 512, 256
NCORES = 8
SS = S // NCORES          # rows of si per core = 8192
P = 128                   # partitions
NT = SS // P              # 128-row tiles per core = 64
NB = 16                   # DMA batches of si per core
TPB = NT // NB            # tiles per batch = 8
HC = H // P               # h chunks per tile = 4
EC = (2 * H) // P         # enc chunks = 8

TRACE = False
LAST_EXEC_NS = None
LAST_RESULT = None


def build_nc(ss=SS, nb=NB):
    nt = ss // P
    tpb = nt // nb
    nc = bacc.Bacc(num_devices=NCORES)

    si_ext = nc.declare_dram_parameter("si", [ss, H], F32, isOutput=False)
    wat_ext = nc.declare_dram_parameter("wat", [H, A], F32, isOutput=False)
    uat_ext = nc.declare_dram_parameter("uat", [2 * H, A], F32, isOutput=False)
    enc_ext = nc.declare_dram_parameter("enc", [2 * H], F32, isOutput=False)
    va_ext = nc.declare_dram_parameter("va", [1, A], F32, isOutput=False)
    alpha_ext = nc.declare_dram_parameter("alpha", [P, nt], F32, isOutput=True)
    ctx_ext = nc.declare_dram_parameter("ctx", [1, H], F32, isOutput=True)

    cc_in = nc.dram_tensor("cc_in", [1, H + 1], F32)
    cc_out = nc.dram_tensor("cc_out", [NCORES, H + 1], F32, addr_space="Shared")

    with TileContext(nc) as tc:
        with (
            tc.tile_pool(name="consts", bufs=1) as consts,
            tc.tile_pool(name="sipool", bufs=1) as sipool,
            tc.tile_pool(name="work", bufs=3) as work,
            tc.tile_pool(name="pt", bufs=4, space="PSUM") as ptp,
            tc.tile_pool(name="pproj", bufs=1, space="PSUM") as pprojp,
            tc.tile_pool(name="pmix", bufs=2, space="PSUM") as pmixp,
        ):
            # ---- constants / prologue -------------------------------------
            ones_row = consts.tile([1, P], BF16)
            nc.vector.memset(ones_row[:], 1.0)
            ones_col = consts.tile([P, 1], F32)
            nc.vector.memset(ones_col[:], 1.0)

            # warm the PE HAM clock gate with a dense dummy matmul burst
            warm_sb = consts.tile([P, 2 * A], BF16)
            nc.vector.memset(warm_sb[:], 0.0)
            pwarm = pmixp.tile([P, 2 * A], F32, tag="mix")
            for w in range(24):
                nc.tensor.matmul(
                    pwarm[:],
                    warm_sb[:, :P],
                    warm_sb[:],
                    start=True,
                    stop=True,
                    skip_group_check=True,
                )

            wat_sb = consts.tile([P, HC * A], BF16)
            nc.gpsimd.dma_start(
                out=wat_sb.rearrange("p (c a) -> p c a", c=HC),
                in_=wat_ext.rearrange("(c p) a -> p c a", p=P),
            )
            uat_sb = consts.tile([P, EC * A], BF16)
            nc.gpsimd.dma_start(
                out=uat_sb.rearrange("p (c a) -> p c a", c=EC),
                in_=uat_ext.rearrange("(c p) a -> p c a", p=P),
            )
            enc_sb = consts.tile([P, EC], BF16)
            nc.gpsimd.dma_start(
                out=enc_sb[:], in_=enc_ext.rearrange("(c p) -> p c", p=P)
            )
            va_sb = consts.tile([1, A], BF16)
            nc.gpsimd.dma_start(out=va_sb[:], in_=va_ext[:])

            ident = consts.tile([P, P], BF16)
            make_identity(nc, ident)

            # ---- stream si in (cast f32->bf16) and xbar-transpose ---------
            si_b = []
            si_dmas = []
            for b in range(nb):
                t_sb = sipool.tile([P, tpb * H], BF16, name=f"si_b{b}")
                dma = nc.gpsimd.dma_start(
                    out=t_sb.rearrange("p (n h) -> p n h", n=tpb),
                    in_=si_ext[b * tpb * P : (b + 1) * tpb * P, :].rearrange(
                        "(n p) h -> p n h", p=P
                    ),
                )
                # chain (depth 2) so batches complete early-to-late instead of
                # round-robining all packets to a simultaneous late finish
                if b >= 3:
                    add_dep_helper(
                        dma.ins, si_dmas[b - 3].ins, True, "si stream order"
                    )
                si_dmas.append(dma)
                si_b.append(t_sb)

            # enc_proj = enc_out @ Ua.T   -> [1, A]
            pencp = pmixp.tile([1, A], F32, tag="mix")
            for c in range(EC):
                nc.tensor.matmul(
                    pencp[:],
                    enc_sb[:, c : c + 1],
                    uat_sb[:, c * A : (c + 1) * A],
                    start=(c == 0),
                    stop=(c == EC - 1),
                )
            encp_row2 = consts.tile([1, 2 * A], BF16)
            nc.vector.tensor_copy(encp_row2[:, :A], pencp[:])
            nc.vector.tensor_copy(encp_row2[:, A:], pencp[:])

            # va broadcast [P, 4A], pre-scaled by 2 (tanh-via-sigmoid)
            pbc2 = pmixp.tile([P, A], F32, tag="mix")
            nc.tensor.matmul(pbc2[:], ones_row[:], va_sb[:])
            va_bc4 = consts.tile([P, 4 * A], BF16)
            for q in range(4):
                nc.vector.tensor_scalar_mul(
                    va_bc4[:, q * A : (q + 1) * A], pbc2[:], 2.0
                )

            e_all = consts.tile([P, nt], F32)
            exp_all = consts.tile([P, nt], BF16)
            zcol = consts.tile([P, 1], F32)
            zparts = consts.tile([P, nb], F32)
            pPa = pmixp.tile([1, H], F32, tag="mix", name="pPa")
            pPb = pmixp.tile([1, H], F32, tag="mix", name="pPb")

            # ---- main loop: scores ----------------------------------------
            # Two tiles per "pair"; their transposes / matmuls alternate
            # between PSUM banks so drains overlap fills.
            th4 = None
            for p in range(nt // 2):
                t0, t1 = 2 * p, 2 * p + 1
                sl = []
                for tt in (t0, t1):
                    b, k = tt // tpb, tt % tpb
                    sl.append(si_b[b][:, k * H : (k + 1) * H])
                siT2 = work.tile([P, 2 * H], BF16, tag="siT2")
                pproj2 = pprojp.tile([P, 4 * A], F32)      # 2 banks
                pTs = []
                for j in (0, 1):
                    pT = ptp.tile([P, H], BF16, tag="pT", name=f"pT{p}_{j}")
                    pTs.append(pT)
                for c in range(HC):
                    for j in (0, 1):
                        nc.tensor.transpose(
                            pTs[j][:, c * P : (c + 1) * P],
                            sl[j][:, c * P : (c + 1) * P],
                            ident[:],
                        )
                nc.scalar.copy(siT2[:, :H], pTs[0][:])
                nc.vector.tensor_copy(siT2[:, H:], pTs[1][:])
                for j in (0, 1):
                    nc.tensor.matmul(
                        pproj2[:, j * 2 * A : j * 2 * A + A],
                        ones_row[:],
                        encp_row2[:, :A],
                        start=True,
                        stop=False,
                        skip_group_check=True,
                    )
                for c in range(HC):
                    for j in (0, 1):
                        nc.tensor.matmul(
                            pproj2[:, j * 2 * A : j * 2 * A + A],
                            siT2[:, j * H + c * P : j * H + (c + 1) * P],
                            wat_sb[:, c * A : (c + 1) * A],
                            start=False,
                            stop=(c == HC - 1),
                            skip_group_check=True,
                        )
                if p % 2 == 0:
                    th4 = work.tile([P, 4 * A], BF16)
                q = p % 2
                nc.scalar.activation(
                    th4[:, q * 2 * A : (q + 1) * 2 * A].rearrange(
                        "p (b x) -> p b x", b=2
                    ),
                    pproj2.rearrange("p (b x) -> p b x", b=2)[:, :, :A],
                    mybir.ActivationFunctionType.Sigmoid,
                    scale=2.0,
                )
                if p % 2 == 1:
                    t = t1
                    prod4 = work.tile([P, 4 * A], BF16)
                    nc.vector.tensor_mul(prod4[:], th4[:], va_bc4[:])
                    nc.vector.reduce_sum(
                        e_all[:, t - 3 : t + 1],
                        prod4.rearrange("p (n a) -> p n a", n=4),
                        axis=mybir.AxisListType.X,
                    )
                    # exp(e) = sigma(e)/(1-sigma(e)) -- same ACT table as the
                    # loop's sigmoid, so no activation-table thrash.
                    bb = t // tpb
                    esl = e_all[:, bb * tpb : (bb + 1) * tpb]
                    sigc = work.tile([P, tpb], F32, tag="sigc")
                    nc.scalar.activation(
                        sigc[:], esl, mybir.ActivationFunctionType.Sigmoid
                    )
                    omc = work.tile([P, tpb], F32, tag="omc")
                    nc.vector.tensor_scalar(
                        out=omc[:],
                        in0=sigc[:],
                        scalar1=-1.0,
                        scalar2=1.0,
                        op0=mybir.AluOpType.mult,
                        op1=mybir.AluOpType.add,
                    )
                    rec = work.tile([P, tpb], F32, tag="rec")
                    nc.vector.reciprocal(rec[:], omc[:])
                    nc.vector.tensor_mul(
                        exp_all[:, bb * tpb : (bb + 1) * tpb], sigc[:], rec[:]
                    )
                    nc.vector.reduce_sum(
                        zparts[:, bb : bb + 1],
                        exp_all[:, bb * tpb : (bb + 1) * tpb],
                        axis=mybir.AxisListType.X,
                    )
                    # weighted-sum pass for this batch (ping-pong psum rows)
                    for tt in range(bb * tpb, (bb + 1) * tpb):
                        kk = tt % tpb
                        pdst = pPa if tt % 2 == 0 else pPb
                        nc.tensor.matmul(
                            pdst[:],
                            exp_all[:, tt : tt + 1],
                            si_b[bb][:, kk * H : (kk + 1) * H],
                            start=(tt < 2),
                            stop=(tt >= nt - 2),
                            skip_group_check=True,
                        )

            # ---- tail: Z, allreduce, outputs ------------------------------
            nc.vector.reduce_sum(
                zcol[:], zparts[:], axis=mybir.AxisListType.X
            )
            pZ = pmixp.tile([1, 1], F32, tag="mix")
            nc.tensor.matmul(pZ[:], zcol[:], ones_col[:])

            cc_sb = consts.tile([1, H + 1], F32)
            nc.vector.tensor_copy(cc_sb[:, :H], pPa[:])
            nc.vector.tensor_add(cc_sb[:, :H], cc_sb[:, :H], pPb[:])
            nc.vector.tensor_copy(cc_sb[:, H : H + 1], pZ[:])
            nc.sync.dma_start(out=cc_in[:], in_=cc_sb[:])
            nc.gpsimd.collective_compute(
                "AllGather",
                mybir.AluOpType.bypass,
                replica_groups=[list(range(NCORES))],
                ins=[cc_in[:]],
                outs=[cc_out[:]],
            )
            ccg_sb = consts.tile([NCORES, H + 1], F32)
            nc.sync.dma_start(out=ccg_sb[:], in_=cc_out[:])
            ones8 = consts.tile([NCORES, 1], F32)
            nc.vector.memset(ones8[:], 1.0)
            psum_cc = pmixp.tile([1, H], F32, tag="mix")
            nc.tensor.matmul(psum_cc[:], ones8[:], ccg_sb[:, :H])
            psum_ccz = pmixp.tile([1, 1], F32, tag="mix")
            nc.tensor.matmul(psum_ccz[:], ones8[:], ccg_sb[:, H:])
            ccr_sb = consts.tile([1, H + 1], F32)
            nc.vector.tensor_copy(ccr_sb[:, :H], psum_cc[:])
            nc.vector.tensor_copy(ccr_sb[:, H:], psum_ccz[:])

            invZ = consts.tile([1, 1], F32)
            nc.vector.reciprocal(invZ[:], ccr_sb[:, H : H + 1])
            ctx_sb = consts.tile([1, H], F32)
            nc.vector.tensor_scalar_mul(ctx_sb[:], ccr_sb[:, :H], invZ[:])
            nc.sync.dma_start(out=ctx_ext[:], in_=ctx_sb[:])

            onesf_row = consts.tile([1, P], F32)
            nc.vector.memset(onesf_row[:], 1.0)
            pbias = pmixp.tile([P, 1], F32, tag="mix")
            nc.tensor.matmul(pbias[:], onesf_row[:], invZ[:])
            invZ_col = consts.tile([P, 1], F32)
            nc.vector.tensor_copy(invZ_col[:], pbias[:])
            alpha_sb = consts.tile([P, nt], F32)
            nc.vector.tensor_scalar(
                out=alpha_sb[:],
                in0=exp_all[:],
                scalar1=invZ_col[:],
                scalar2=None,
                op0=mybir.AluOpType.mult,
            )
            nc.sync.dma_start(out=alpha_ext[:], in_=alpha_sb[:])

    nc.compile()
    return nc


_NC_CACHE = None


def kernel(enc_out, si, Wa, Ua, va):
    global LAST_EXEC_NS, LAST_RESULT, _NC_CACHE

    enc_out = np.ascontiguousarray(np.asarray(enc_out, dtype=np.float32))
    si = np.ascontiguousarray(np.asarray(si, dtype=np.float32))
    Wa = np.asarray(Wa, dtype=np.float32)
    Ua = np.asarray(Ua, dtype=np.float32)
    va = np.asarray(va, dtype=np.float32)

    si2 = si.reshape(S, H)
    wat = np.ascontiguousarray(Wa.T)          # [H, A]
    uat = np.ascontiguousarray(Ua.T)          # [2H, A]
    enc = np.ascontiguousarray(enc_out.reshape(2 * H))
    va_row = np.ascontiguousarray(va.reshape(1, A))

    if _NC_CACHE is None:
        _NC_CACHE = build_nc()
    nc = _NC_CACHE

    in_maps = []
    for i in range(NCORES):
        in_maps.append(
            {
                "si": np.ascontiguousarray(si2[i * SS : (i + 1) * SS, :]),
                "wat": wat,
                "uat": uat,
                "enc": enc,
                "va": va_row,
            }
        )

    res = run_bass_kernel_spmd(nc, in_maps, list(range(NCORES)), trace=TRACE)
    LAST_EXEC_NS = res.exec_time_ns
    LAST_RESULT = res

    alpha_full = np.empty((S,), dtype=np.float32)
    for i in range(NCORES):
        a = np.asarray(res.results[i]["alpha"])          # [P, NT]
        alpha_full[i * SS : (i + 1) * SS] = a.T.reshape(SS)
    context = np.asarray(res.results[0]["ctx"]).reshape(1, H)
    return context, alpha_full.reshape(1, S, 1)


# revision 25
# speedup vs baseline: 1.0401x; 1.0401x over previous
"""Bahdanau additive attention on 8 TRN2 NeuronCores.

Reference computation (B=1, S=65536, H=512, A=256):
    si_proj = si @ Wa.T                       [S, A]
    enc_proj = enc_out @ Ua.T                 [A]
    e = tanh(si_proj + enc_proj) @ va         [S]
    alpha = softmax(e)                        [S]
    context = alpha @ si                      [H]
    returns (context [1,H], alpha [1,S,1])

Distribution: sequence-parallel over S. Each core owns 8192 rows of si,
computes its local unnormalized exp(e) and partial weighted sums, then one
513-float AllGather (+local sum) combines the softmax normalizer Z and
partial context P.
alpha = exp(e - ln Z) and context = P/Z are finalized on-device.

softmax max-subtraction is skipped: |e| <= sum|va| ~ 13, exp(13)*65536 fits
comfortably in f32.
"""

import sys
import numpy as np

for _p in ("/opt/trn_rl_repo",):
    if _p not in sys.path:
        sys.path.insert(0, _p)

import concourse.bacc as bacc
import concourse.mybir as mybir
from concourse.tile import TileContext
from concourse.tile_rust import add_dep_helper
from concourse.bass_utils import run_bass_kernel_spmd
from concourse.masks import make_identity

F32 = mybir.dt.float32
BF16 = mybir.dt.bfloat16

B, S, H, A = 1, 65536, 512, 256
NCORES = 8
SS = S // NCORES          # rows of si per core = 8192
P = 128                   # partitions
NT = SS // P              # 128-row tiles per core = 64
NB = 16                   # DMA batches of si per core
TPB = NT // NB            # tiles per batch = 8
HC = H // P               # h chunks per tile = 4
EC = (2 * H) // P         # enc chunks = 8

TRACE = False
LAST_EXEC_NS = None
LAST_RESULT = None


def build_nc(ss=SS, nb=NB):
    nt = ss // P
    tpb = nt // nb
    nc = bacc.Bacc(num_devices=NCORES)

    si_ext = nc.declare_dram_parameter("si", [ss, H], F32, isOutput=False)
    wat_ext = nc.declare_dram_parameter("wat", [H, A], F32, isOutput=False)
    uat_ext = nc.declare_dram_parameter("uat", [2 * H, A], F32, isOutput=False)
    enc_ext = nc.declare_dram_parameter("enc", [2 * H], F32, isOutput=False)
    va_ext = nc.declare_dram_parameter("va", [1, A], F32, isOutput=False)
    alpha_ext = nc.declare_dram_parameter("alpha", [P, nt], F32, isOutput=True)
    ctx_ext = nc.declare_dram_parameter("ctx", [1, H], F32, isOutput=True)

    cc_in = nc.dram_tensor("cc_in", [1, H + 1], F32)
    cc_out = nc.dram_tensor("cc_out", [NCORES, H + 1], F32, addr_space="Shared")

    with TileContext(nc) as tc:
        with (
            tc.tile_pool(name="consts", bufs=1) as consts,
            tc.tile_pool(name="sipool", bufs=1) as sipool,
            tc.tile_pool(name="work", bufs=3) as work,
            tc.tile_pool(name="pt", bufs=4, space="PSUM") as ptp,
            tc.tile_pool(name="pproj", bufs=1, space="PSUM") as pprojp,
            tc.tile_pool(name="pmix", bufs=2, space="PSUM") as pmixp,
        ):
            # ---- constants / prologue -------------------------------------
            ones_row = consts.tile([1, P], BF16)
            nc.vector.memset(ones_row[:], 1.0)
            ones_col = consts.tile([P, 1], F32)
            nc.vector.memset(ones_col[:], 1.0)

            # warm the PE HAM clock gate with a dense dummy matmul burst
            warm_sb = consts.tile([P, 2 * A], BF16)
            nc.vector.memset(warm_sb[:], 0.0)
            pwarm = pmixp.tile([P, 2 * A], F32, tag="mix")
            for w in range(24):
                nc.tensor.matmul(
                    pwarm[:],
                    warm_sb[:, :P],
                    warm_sb[:],
                    start=True,
                    stop=True,
                    skip_group_check=True,
                )

            wat_sb = consts.tile([P, HC * A], BF16)
            nc.gpsimd.dma_start(
                out=wat_sb.rearrange("p (c a) -> p c a", c=HC),
                in_=wat_ext.rearrange("(c p) a -> p c a", p=P),
            )
            uat_sb = consts.tile([P, EC * A], BF16)
            nc.gpsimd.dma_start(
                out=uat_sb.rearrange("p (c a) -> p c a", c=EC),
                in_=uat_ext.rearrange("(c p) a -> p c a", p=P),
            )
            enc_sb = consts.tile([P, EC], BF16)
            nc.gpsimd.dma_start(
                out=enc_sb[:], in_=enc_ext.rearrange("(c p) -> p c", p=P)
            )
            va_sb = consts.tile([1, A], BF16)
            nc.gpsimd.dma_start(out=va_sb[:], in_=va_ext[:])

            ident = consts.tile([P, P], BF16)
            make_identity(nc, ident)

            # ---- stream si in (cast f32->bf16) and xbar-transpose ---------
            si_b = []
            si_dmas = []
            for b in range(nb):
                t_sb = sipool.tile([P, tpb * H], BF16, name=f"si_b{b}")
                dma = nc.gpsimd.dma_start(
                    out=t_sb.rearrange("p (n h) -> p n h", n=tpb),
                    in_=si_ext[b * tpb * P : (b + 1) * tpb * P, :].rearrange(
                        "(n p) h -> p n h", p=P
                    ),
                )
                # chain (depth 2) so batches complete early-to-late instead of
                # round-robining all packets to a simultaneous late finish
                if b >= 2:
                    add_dep_helper(
                        dma.ins, si_dmas[b - 2].ins, True, "si stream order"
                    )
                si_dmas.append(dma)
                si_b.append(t_sb)

            # enc_proj = enc_out @ Ua.T   -> [1, A]
            pencp = pmixp.tile([1, A], F32, tag="mix")
            for c in range(EC):
                nc.tensor.matmul(
                    pencp[:],
                    enc_sb[:, c : c + 1],
                    uat_sb[:, c * A : (c + 1) * A],
                    start=(c == 0),
                    stop=(c == EC - 1),
                )
            encp_row2 = consts.tile([1, 2 * A], BF16)
            nc.vector.tensor_copy(encp_row2[:, :A], pencp[:])
            nc.vector.tensor_copy(encp_row2[:, A:], pencp[:])

            # va broadcast [P, 4A], pre-scaled by 2 (tanh-via-sigmoid)
            pbc2 = pmixp.tile([P, A], F32, tag="mix")
            nc.tensor.matmul(pbc2[:], ones_row[:], va_sb[:])
            va_bc4 = consts.tile([P, 4 * A], BF16)
            for q in range(4):
                nc.vector.tensor_scalar_mul(
                    va_bc4[:, q * A : (q + 1) * A], pbc2[:], 2.0
                )

            e_all = consts.tile([P, nt], F32)
            exp_all = consts.tile([P, nt], BF16)
            zcol = consts.tile([P, 1], F32)
            zparts = consts.tile([P, nb], F32)
            pPa = pmixp.tile([1, H], F32, tag="mix", name="pPa")
            pPb = pmixp.tile([1, H], F32, tag="mix", name="pPb")

            # ---- main loop: scores ----------------------------------------
            # Two tiles per "pair"; their transposes / matmuls alternate
            # between PSUM banks so drains overlap fills.
            th4 = None
            for p in range(nt // 2):
                t0, t1 = 2 * p, 2 * p + 1
                sl = []
                for tt in (t0, t1):
                    b, k = tt // tpb, tt % tpb
                    sl.append(si_b[b][:, k * H : (k + 1) * H])
                siT2 = work.tile([P, 2 * H], BF16, tag="siT2")
                pproj2 = pprojp.tile([P, 4 * A], F32)      # 2 banks
                pTs = []
                for j in (0, 1):
                    pT = ptp.tile([P, H], BF16, tag="pT", name=f"pT{p}_{j}")
                    pTs.append(pT)
                for c in range(HC):
                    for j in (0, 1):
                        nc.tensor.transpose(
                            pTs[j][:, c * P : (c + 1) * P],
                            sl[j][:, c * P : (c + 1) * P],
                            ident[:],
                        )
                nc.scalar.copy(siT2[:, :H], pTs[0][:])
                nc.vector.tensor_copy(siT2[:, H:], pTs[1][:])
                for j in (0, 1):
                    nc.tensor.matmul(
                        pproj2[:, j * 2 * A : j * 2 * A + A],
                        ones_row[:],
                        encp_row2[:, :A],
                        start=True,
                        stop=False,
                        skip_group_check=True,
                    )
                for c in range(HC):
                    for j in (0, 1):
                        nc.tensor.matmul(
                            pproj2[:, j * 2 * A : j * 2 * A + A],
                            siT2[:, j * H + c * P : j * H + (c + 1) * P],
                            wat_sb[:, c * A : (c + 1) * A],
                            start=False,
                            stop=(c == HC - 1),
                            skip_group_check=True,
                        )
                if p % 2 == 0:
                    th4 = work.tile([P, 4 * A], BF16)
                q = p % 2
                nc.scalar.activation(
                    th4[:, q * 2 * A : (q + 1) * 2 * A].rearrange(
                        "p (b x) -> p b x", b=2
                    ),
                    pproj2.rearrange("p (b x) -> p b x", b=2)[:, :, :A],
                    mybir.ActivationFunctionType.Sigmoid,
                    scale=2.0,
                )
                if p % 2 == 1:
                    t = t1
                    prod4 = work.tile([P, 4 * A], BF16)
                    nc.vector.tensor_mul(prod4[:], th4[:], va_bc4[:])
                    nc.vector.reduce_sum(
                        e_all[:, t - 3 : t + 1],
                        prod4.rearrange("p (n a) -> p n a", n=4),
                        axis=mybir.AxisListType.X,
                    )
                    # exp(e) = sigma(e)/(1-sigma(e)) -- same ACT table as the
                    # loop's sigmoid, so no activation-table thrash.
                    bb = t // tpb
                    esl = e_all[:, bb * tpb : (bb + 1) * tpb]
                    sigc = work.tile([P, tpb], F32, tag="sigc")
                    nc.scalar.activation(
                        sigc[:], esl, mybir.ActivationFunctionType.Sigmoid
                    )
                    omc = work.tile([P, tpb], F32, tag="omc")
                    nc.vector.tensor_scalar(
                        out=omc[:],
                        in0=sigc[:],
                        scalar1=-1.0,
                        scalar2=1.0,
                        op0=mybir.AluOpType.mult,
                        op1=mybir.AluOpType.add,
                    )
                    rec = work.tile([P, tpb], F32, tag="rec")
                    nc.vector.reciprocal(rec[:], omc[:])
                    nc.vector.tensor_mul(
                        exp_all[:, bb * tpb : (bb + 1) * tpb], sigc[:], rec[:]
                    )
                    nc.vector.reduce_sum(
                        zparts[:, bb : bb + 1],
                        exp_all[:, bb * tpb : (bb + 1) * tpb],
                        axis=mybir.AxisListType.X,
                    )
                    # weighted-sum pass for this batch (ping-pong psum rows)
                    for tt in range(bb * tpb, (bb + 1) * tpb):
                        kk = tt % tpb
                        pdst = pPa if tt % 2 == 0 else pPb
                        nc.tensor.matmul(
                            pdst[:],
                            exp_all[:, tt : tt + 1],
                            si_b[bb][:, kk * H : (kk + 1) * H],
                            start=(tt < 2),
                            stop=(tt >= nt - 2),
                            skip_group_check=True,
                        )

            # ---- tail: Z, allreduce, outputs ------------------------------
            nc.vector.reduce_sum(
                zcol[:], zparts[:], axis=mybir.AxisListType.X
            )
            pZ = pmixp.tile([1, 1], F32, tag="mix")
            nc.tensor.matmul(pZ[:], zcol[:], ones_col[:])

            cc_sb = consts.tile([1, H + 1], F32)
            nc.vector.tensor_copy(cc_sb[:, :H], pPa[:])
            nc.vector.tensor_add(cc_sb[:, :H], cc_sb[:, :H], pPb[:])
            nc.vector.tensor_copy(cc_sb[:, H : H + 1], pZ[:])
            nc.sync.dma_start(out=cc_in[:], in_=cc_sb[:])
            nc.gpsimd.collective_compute(
                "AllGather",
                mybir.AluOpType.bypass,
                replica_groups=[list(range(NCORES))],
                ins=[cc_in[:]],
                outs=[cc_out[:]],
            )
            ccg_sb = consts.tile([NCORES, H + 1], F32)
            nc.sync.dma_start(out=ccg_sb[:], in_=cc_out[:])
            ones8 = consts.tile([NCORES, 1], F32)
            nc.vector.memset(ones8[:], 1.0)
            psum_cc = pmixp.tile([1, H], F32, tag="mix")
            nc.tensor.matmul(psum_cc[:], ones8[:], ccg_sb[:, :H])
            psum_ccz = pmixp.tile([1, 1], F32, tag="mix")
            nc.tensor.matmul(psum_ccz[:], ones8[:], ccg_sb[:, H:])
            ccr_sb = consts.tile([1, H + 1], F32)
            nc.vector.tensor_copy(ccr_sb[:, :H], psum_cc[:])
            nc.vector.tensor_copy(ccr_sb[:, H:], psum_ccz[:])

            invZ = consts.tile([1, 1], F32)
            nc.vector.reciprocal(invZ[:], ccr_sb[:, H : H + 1])
            ctx_sb = consts.tile([1, H], F32)
            nc.vector.tensor_scalar_mul(ctx_sb[:], ccr_sb[:, :H], invZ[:])
            nc.sync.dma_start(out=ctx_ext[:], in_=ctx_sb[:])

            onesf_row = consts.tile([1, P], F32)
            nc.vector.memset(onesf_row[:], 1.0)
            pbias = pmixp.tile([P, 1], F32, tag="mix")
            nc.tensor.matmul(pbias[:], onesf_row[:], invZ[:])
            invZ_col = consts.tile([P, 1], F32)
            nc.vector.tensor_copy(invZ_col[:], pbias[:])
            alpha_sb = consts.tile([P, nt], F32)
            nc.vector.tensor_scalar(
                out=alpha_sb[:],
                in0=exp_all[:],
                scalar1=invZ_col[:],
                scalar2=None,
                op0=mybir.AluOpType.mult,
            )
            nc.sync.dma_start(out=alpha_ext[:], in_=alpha_sb[:])

    nc.compile()
    return nc


_NC_CACHE = None


def kernel(enc_out, si, Wa, Ua, va):
    global LAST_EXEC_NS, LAST_RESULT, _NC_CACHE

    enc_out = np.ascontiguousarray(np.asarray(enc_out, dtype=np.float32))
    si = np.ascontiguousarray(np.asarray(si, dtype=np.float32))
    Wa = np.asarray(Wa, dtype=np.float32)
    Ua = np.asarray(Ua, dtype=np.float32)
    va = np.asarray(va, dtype=np.float32)

    si2 = si.reshape(S, H)
    wat = np.ascontiguousarray(Wa.T)          # [H, A]
    uat = np.ascontiguousarray(Ua.T)          # [2H, A]
    enc = np.ascontiguousarray(enc_out.reshape(2 * H))
    va_row = np.ascontiguousarray(va.reshape(1, A))

    if _NC_CACHE is None:
        _NC_CACHE = build_nc()
    nc = _NC_CACHE

    in_maps = []
    for i in range(NCORES):
        in_maps.append(
            {
                "si": np.ascontiguousarray(si2[i * SS : (i + 1) * SS, :]),
                "wat": wat,
                "uat": uat,
                "enc": enc,
                "va": va_row,
            }
        )

    res = run_bass_kernel_spmd(nc, in_maps, list(range(NCORES)), trace=TRACE)
    LAST_EXEC_NS = res.exec_time_ns
    LAST_RESULT = res

    alpha_full = np.empty((S,), dtype=np.float32)
    for i in range(NCORES):
        a = np.asarray(res.results[i]["alpha"])          # [P, NT]
        alpha_full[i * SS : (i + 1) * SS] = a.T.reshape(SS)
    context = np.asarray(res.results[0]["ctx"]).reshape(1, H)
    return context, alpha_full.reshape(1, S, 1)
